# revision 9
# baseline (speedup 1.0000x reference)
"""CrossAttention kernel for 8 Trainium2 NeuronCores (Bass/Tile).

Problem (hardcoded): x [4,2048,1024] f32, context [4,2048,1024] f32,
mask [4,2048] bool, Wq/Wk/Wv [1024,512], Wo [512,1024], bo [1024].
8 heads x 64 dim, scale 1/8, out = softmax(q k^T * s + maskbias) v @ Wo + bo.

Sharding: core c -> (batch b = c//2, head-group hg = c%2 of 4 heads).
Each core computes a partial output [2048,1024] (its 4 heads through its
256-row slice of Wo); the host sums core pairs and adds bo.

Device-side layout trick: everything is computed in "transposed" form so
no on-device transposes are needed:
  qT/kT = W^T @ x^T come out of the projection matmul as [d, rows].
  sim is computed as simT [j, i]  (lhsT=kT tile, rhs=qT tile), so the
  attention scale fuses into the ACT exp (exp(sim*scale)).
  PV uses expT directly as the moving operand with v' = [v | ones] as the
  stationary one; the ones column yields the softmax denominator for free.
  The PV output [d, i] is exactly the lhsT the Wo projection needs.

Mask handling is free: the host drops masked context rows (softmax weight
exactly zero) and pads to a multiple of 128 with all-zero context rows.
Zero context rows give k=0 -> sim=0 -> exp=1, and the ones column is 0 on
pad rows (host-supplied), so pads contribute nothing to numerator or
denominator. No bias tensor, no ACT bias read.

Steady-state pipelining (the body is PE-bound at ~104us of matmul):
qT/kT/vp/oT are double-buffered by body parity, so body r+1's q/k/v
projections have no WAR hazard against body r and are enqueued as PE
filler at body r's start. One persistent filler queue spans all bodies
(no per-body drain barrier); the out-projection for i-slice ic is
front-queued when its block retires and drains as filler during the
following blocks/body. attn() force-runs (ensure) any still-queued
generator it is about to consume, so emission order stays legal no
matter how far the pump lags.
"""

import math

import numpy as np
import ml_dtypes

BF16 = ml_dtypes.bfloat16

B, N, DIM = 4, 2048, 1024
HEADS, DH = 8, 64
INNER = HEADS * DH  # 512
HG = INNER // 2  # 256 per head-group

_PROGRAMS: dict[tuple, object] = {}

# Diagnostic hook: test tooling can monkeypatch instruction-name generation
# and read this label to attribute instructions to emission sites.
_LABEL = ["?"]


def _build_program(m_pad: int, repeats: int = 1):
    import collections

    import concourse.tile as tile
    from concourse import bacc, mybir

    f32 = mybir.dt.float32
    bf16 = mybir.dt.bfloat16
    Exp = mybir.ActivationFunctionType.Exp
    mpt = m_pad // 128

    nc = bacc.Bacc("TRN2", target_bir_lowering=False, debug=False)
    xT_d = nc.dram_tensor("xT", [DIM, N], bf16, kind="ExternalInput").ap()
    cT_d = nc.dram_tensor("ctxT", [DIM, m_pad], bf16, kind="ExternalInput").ap()
    wq_d = nc.dram_tensor("wq", [DIM, HG], bf16, kind="ExternalInput").ap()
    wk_d = nc.dram_tensor("wk", [DIM, HG], bf16, kind="ExternalInput").ap()
    wv_d = nc.dram_tensor("wv", [DIM, HG], bf16, kind="ExternalInput").ap()
    wo_d = nc.dram_tensor("wo", [HG, DIM], bf16, kind="ExternalInput").ap()
    ones_d = nc.dram_tensor("ones", [128, mpt], bf16, kind="ExternalInput").ap()
    out_d = nc.dram_tensor("out", [N, DIM], f32, kind="ExternalOutput").ap()

    nsides = 2 if repeats > 1 else 1

    with tile.TileContext(nc) as tc:
        with tc.tile_pool(name="const", bufs=1) as const, tc.tile_pool(
            name="work", bufs=4
        ) as work, tc.tile_pool(name="outp", bufs=3) as outp:
            xT = const.tile([128, 8, N], bf16)
            cT = const.tile([128, 8, m_pad], bf16)
            wq = const.tile([128, 8, HG], bf16)
            wk = const.tile([128, 8, HG], bf16)
            wv = const.tile([128, 8, HG], bf16)
            wo = const.tile([128, 2, DIM], bf16)
            # double-buffered by body parity (dim 1 = side)
            qT = const.tile([128, nsides, 2, N], bf16)
            kT = const.tile([128, nsides, 2, m_pad], bf16)
            vp = const.tile([128, nsides, mpt, 4, DH + 1], bf16)
            oT = const.tile([128, nsides, 2, N], bf16)

            # DMA order matters for the pipeline head: weights first (tiny,
            # unblock the projection matmuls), then context (v/k-proj), then
            # x (q-proj is needed later than v/k).
            for kt in range(8):
                s = slice(kt * 128, (kt + 1) * 128)
                nc.sync.dma_start(out=wv[:, kt, :], in_=wv_d[s, :])
                nc.sync.dma_start(out=wk[:, kt, :], in_=wk_d[s, :])
                nc.sync.dma_start(out=wq[:, kt, :], in_=wq_d[s, :])
            for sd in range(nsides):
                for lh in range(4):
                    nc.sync.dma_start(
                        out=vp[:, sd, :, lh, DH], in_=ones_d[:, :]
                    )
            for kt in range(8):
                s = slice(kt * 128, (kt + 1) * 128)
                nc.sync.dma_start(out=cT[:, kt, :], in_=cT_d[s, :])
            for kt in range(8):
                s = slice(kt * 128, (kt + 1) * 128)
                nc.sync.dma_start(out=xT[:, kt, :], in_=xT_d[s, :])
            nc.sync.dma_start(out=wo[:, 0, :], in_=wo_d[0:128, :])
            nc.sync.dma_start(out=wo[:, 1, :], in_=wo_d[128:256, :])

            kchunks = []
            j0 = 0
            while j0 < m_pad:
                jl = min(512, m_pad - j0)
                kchunks.append((slice(j0, j0 + jl), jl))
                j0 += jl

            with tc.tile_pool(name="ps", bufs=2, space="PSUM") as psp:
                # ---- persistent filler machinery ----------------------
                # One queue spans all bodies. pump(n) advances queued
                # generators by ~n matmuls; ensure(key) force-finishes a
                # specific generator so attn can consume its output with
                # correct emission order even when the pump lags.
                gens: dict[tuple, object] = {}
                fifo = collections.deque()
                budget = [0]

                def enqueue(key, g, front=False):
                    gens[key] = g
                    if front:
                        fifo.appendleft(key)
                    else:
                        fifo.append(key)

                def pump(nmm):
                    budget[0] += nmm
                    while fifo and budget[0] > 0:
                        key = fifo[0]
                        g = gens.get(key)
                        if g is None:
                            fifo.popleft()
                            continue
                        try:
                            budget[0] -= next(g)
                        except StopIteration:
                            del gens[key]
                            fifo.popleft()

                def ensure(key):
                    g = gens.pop(key, None)
                    if g is None:
                        return
                    for _ in g:
                        pass

                def drain_all():
                    budget[0] = 0
                    while fifo:
                        key = fifo.popleft()
                        g = gens.pop(key, None)
                        if g is None:
                            continue
                        for _ in g:
                            pass

                # ---- generators ---------------------------------------
                def gen_proj(w, sd, pr, dst, cs, src, jl):
                    # dst[:, sd, pr, cs] = (w[:,:,128pr:])^T @ src[:, :, cs]
                    ws = slice(pr * 128, (pr + 1) * 128)
                    ps = psp.tile([128, 512], f32, tag="proj", name="ps")
                    for kt in range(8):
                        _LABEL[0] = f"proj{'q' if w is wq else 'k'}:{sd}:{pr}"
                        nc.tensor.matmul(
                            ps[:, :jl],
                            lhsT=w[:, kt, ws],
                            rhs=src[:, kt, cs],
                            start=(kt == 0),
                            stop=(kt == 7),
                        )
                        if kt % 2 == 1:
                            yield 2
                    nc.vector.tensor_copy(
                        out=dst[:, sd, pr, cs], in_=ps[:, :jl]
                    )
                    yield 0

                def gen_vproj(sd, jt):
                    js = slice(jt * 128, (jt + 1) * 128)
                    ps = psp.tile([128, 4, DH], f32, tag="proj", name="psv")
                    for kt in range(8):
                        _LABEL[0] = f"projv:{sd}:{jt}"
                        nc.tensor.matmul(
                            ps[:, :, :],
                            lhsT=cT[:, kt, js],
                            rhs=wv[:, kt, :],
                            start=(kt == 0),
                            stop=(kt == 7),
                        )
                        if kt % 2 == 1:
                            yield 2
                    nc.vector.tensor_copy(
                        out=vp[:, sd, jt, :, 0:DH], in_=ps[:, :, :]
                    )
                    yield 0

                def gen_po(sd, ic):
                    # ---- output projection for i-slice ic -----------------
                    # Pure filler: front-queued when block (pr=1, ic)
                    # retires; by pump time its inputs (normalized oT rows)
                    # are ready, and the 8 MB writeback DMA overlaps
                    # attention compute.
                    for it in range(ic * 4, ic * 4 + 4):
                        _LABEL[0] = f"po:{sd}:{ic}"
                        ts_ = slice(it * 128, (it + 1) * 128)
                        ob = outp.tile([128, DIM], f32, tag="ob", name="ob")
                        for nh2 in range(2):
                            ns = slice(nh2 * 512, (nh2 + 1) * 512)
                            ph = psp.tile(
                                [128, 512], f32, tag="proj", name="ph"
                            )
                            for ck2 in range(2):
                                nc.tensor.matmul(
                                    ph[:, :],
                                    lhsT=oT[:, sd, ck2, ts_],
                                    rhs=wo[:, ck2, ns],
                                    start=(ck2 == 0),
                                    stop=(ck2 == 1),
                                )
                            yield 2
                            nc.vector.tensor_copy(out=ob[:, ns], in_=ph[:, :])
                            nc.sync.dma_start(
                                out=out_d[ts_, ns], in_=ob[:, ns]
                            )
                        yield 0

                def emit_projections(sd):
                    # Enqueue body sd's q/k/v projections in the order the
                    # attention blocks will need them.
                    for ci, (cs, jl) in enumerate(kchunks):
                        enqueue(
                            ("k", sd, 0, ci),
                            gen_proj(wk, sd, 0, kT, cs, cT, jl),
                        )
                    enqueue(
                        ("q", sd, 0, 0),
                        gen_proj(wq, sd, 0, qT, slice(0, 512), xT, 512),
                    )
                    for jt in range(mpt):
                        enqueue(("v", sd, jt), gen_vproj(sd, jt))
                    for icc in range(1, N // 512):
                        cs = slice(icc * 512, (icc + 1) * 512)
                        enqueue(
                            ("q", sd, 0, icc),
                            gen_proj(wq, sd, 0, qT, cs, xT, 512),
                        )
                    for ci, (cs, jl) in enumerate(kchunks):
                        enqueue(
                            ("k", sd, 1, ci),
                            gen_proj(wk, sd, 1, kT, cs, cT, jl),
                        )
                    for icc in range(N // 512):
                        cs = slice(icc * 512, (icc + 1) * 512)
                        enqueue(
                            ("q", sd, 1, icc),
                            gen_proj(wq, sd, 1, qT, cs, xT, 512),
                        )

                # ---- attention ----------------------------------------
                def attn(sd, pr, ic):
                    # ---- attention for head pair (2pr, 2pr+1) -------------
                    # The two K=64 sim matmuls use PE row-groups 0-1 / 2-3
                    # (auto tile_position from lhsT base partition 0/64),
                    # writing the two 512-halves (= 2 banks) of one psum
                    # tile; one ACT exp covers both heads.
                    if ic == 0:
                        for ci in range(len(kchunks)):
                            ensure(("k", sd, pr, ci))
                    ensure(("q", sd, pr, ic))
                    _LABEL[0] = f"attn:{sd}:{pr}:{ic}"
                    i0 = ic * 512
                    qs = slice(i0, i0 + 512)
                    acc0 = psp.tile([65, 512], f32, tag="acc0", bufs=1)
                    acc1 = psp.tile([65, 512], f32, tag="acc1", bufs=1)
                    for jt in range(mpt):
                        if pr == 0 and ic == 0:
                            ensure(("v", sd, jt))
                        _LABEL[0] = f"attn:{sd}:{pr}:{ic}"
                        js = slice(jt * 128, (jt + 1) * 128)
                        sim = psp.tile([128, 1024], f32, tag="sim")
                        nc.tensor.matmul(
                            sim[:, 0:512],
                            lhsT=kT[0:64, sd, pr, js],
                            rhs=qT[0:64, sd, pr, qs],
                            start=True,
                            stop=True,
                        )
                        nc.tensor.matmul(
                            sim[:, 512:1024],
                            lhsT=kT[64:128, sd, pr, js],
                            rhs=qT[64:128, sd, pr, qs],
                            start=True,
                            stop=True,
                        )
                        ex = work.tile([128, 1024], bf16, tag="exp", bufs=6)
                        nc.scalar.activation(
                            out=ex[:, :],
                            in_=sim[:, :],
                            func=Exp,
                            scale=0.125,
                        )
                        if jt < mpt - 1:
                            nc.tensor.matmul(
                                acc0[:, :],
                                lhsT=vp[:, sd, jt, 2 * pr, :],
                                rhs=ex[:, 0:512],
                                start=(jt == 0),
                                stop=False,
                            )
                            nc.tensor.matmul(
                                acc1[:, :],
                                lhsT=vp[:, sd, jt, 2 * pr + 1, :],
                                rhs=ex[:, 512:1024],
                                start=(jt == 0),
                                stop=False,
                            )
                        else:
                            last_ex = ex
                        pump(5 if jt == 0 else 3)
                    finish(sd, pr, ic, qs, acc0, acc1, last_ex)

                def finish(sd, pr, ic, qs, acc0, acc1, ex):
                    _LABEL[0] = f"fin:{sd}:{pr}:{ic}"
                    jt = mpt - 1
                    nc.tensor.matmul(
                        acc0[:, :],
                        lhsT=vp[:, sd, jt, 2 * pr, :],
                        rhs=ex[:, 0:512],
                        start=(jt == 0),
                        stop=True,
                    )
                    nc.tensor.matmul(
                        acc1[:, :],
                        lhsT=vp[:, sd, jt, 2 * pr + 1, :],
                        rhs=ex[:, 512:1024],
                        start=(jt == 0),
                        stop=True,
                    )
                    # normalize: oT = acc[0:64] * (1/acc[64]) bcast.
                    # Evacuate the accs PSUM->SBUF promptly: this frees the
                    # acc banks, unblocking the next block's PV chain, and
                    # moves the whole normalize (fast reciprocal -> gpsimd
                    # broadcast -> muls) off the critical path into
                    # SBUF-only ops.
                    sc = work.tile([65, 1024], f32, tag="scc")
                    nc.vector.tensor_copy(out=sc[:, 0:512], in_=acc0[:, :])
                    nc.vector.tensor_copy(out=sc[:, 512:1024], in_=acc1[:, :])
                    # The exact DVE reciprocal is iterative (~8 cycles/elem)
                    # and costs free-size x 8 cycles regardless of partition
                    # count, so reshape the [1,1024] den row onto 64
                    # partitions (free-size 16) via a tiny SBUF DMA first.
                    d64 = work.tile([64, 16], f32, tag="d64")
                    nc.sync.dma_start(out=d64[:, :], in_=sc[64:65, :])
                    r64 = work.tile([64, 16], f32, tag="r64")
                    nc.vector.reciprocal(out=r64[:, :], in_=d64[:, :])
                    rc = work.tile([1, 1024], f32, tag="recip")
                    nc.sync.dma_start(out=rc[:, :], in_=r64[:, :])
                    bc = work.tile([64, 1024], f32, tag="bcast")
                    nc.gpsimd.partition_broadcast(bc[:, :], rc[:, :])
                    # The muls are SBUF-only, so they can run on GpSimd; that
                    # keeps the DVE queue short for the latency-critical PSUM
                    # evacuations (proj ring rotation feeds the PE fillers).
                    nc.gpsimd.tensor_mul(
                        oT[0:64, sd, pr, qs], sc[0:64, 0:512], bc[:, 0:512]
                    )
                    st = work.tile([64, 512], bf16, tag="stage")
                    nc.gpsimd.tensor_mul(
                        st[:, :], sc[0:64, 512:1024], bc[:, 512:1024]
                    )
                    nc.sync.dma_start(out=oT[64:128, sd, pr, qs], in_=st[:, :])

                # ---- drive all bodies through one filler stream -------
                for r in range(repeats):
                    sd = r % nsides
                    if r == 0:
                        emit_projections(0)
                    if r + 1 < repeats:
                        emit_projections((r + 1) % nsides)
                    # Emission-order safety: po(r-2) reads oT[sd], which this
                    # body's finish() overwrites; force it out of the queue
                    # (no-op when the pump already drained it).
                    for ic in range(N // 512):
                        ensure(("po", r - 2, ic))
                    for pr in range(2):
                        for ic in range(N // 512):
                            attn(sd, pr, ic)
                            if pr == 1:
                                # Back of the queue: po(r, ic) drains as
                                # filler one body later, when its oT inputs
                                # are long since normalized — front-queuing
                                # it head-of-line-blocks the PE stream for
                                # ~5us behind the normalize chain.
                                enqueue(("po", r, ic), gen_po(sd, ic))
                drain_all()

    nc.compile()
    return nc


def _get_program(m_pad: int, repeats: int = 1):
    key = (m_pad, repeats)
    if key not in _PROGRAMS:
        _PROGRAMS[key] = _build_program(m_pad, repeats)
    return _PROGRAMS[key]


def make_in_maps(x, context, mask, Wq, Wk, Wv, Wo):
    """Host-side sharding: returns (m_pad, list of 8 per-core input dicts)."""
    x = np.asarray(x, dtype=np.float32)
    context = np.asarray(context, dtype=np.float32)
    mask = np.asarray(mask)
    idxs = []
    for b in range(B):
        idx = np.nonzero(mask[b])[0]
        if idx.size == 0:
            # all masked -> reference softmax degenerates to uniform over all
            idx = np.arange(context.shape[1])
        idxs.append(idx)
    m_pad = max(128, 128 * math.ceil(max(i.size for i in idxs) / 128))

    wq8 = np.asarray(Wq, dtype=np.float32)
    wk8 = np.asarray(Wk, dtype=np.float32)
    wv8 = np.asarray(Wv, dtype=np.float32)
    wo8 = np.asarray(Wo, dtype=np.float32)

    in_maps = []
    for c in range(8):
        b, hg = c // 2, c % 2
        idx = idxs[b]
        mb = idx.size
        xT = np.ascontiguousarray(x[b].T).astype(BF16)
        cTt = np.zeros((DIM, m_pad), dtype=BF16)
        cTt[:, :mb] = np.ascontiguousarray(context[b][idx].T)
        onesv = np.zeros((m_pad,), dtype=np.float32)
        onesv[:mb] = 1.0
        ones_t = np.ascontiguousarray(onesv.reshape(m_pad // 128, 128).T)
        s = slice(hg * HG, (hg + 1) * HG)
        in_maps.append(
            {
                "xT": xT,
                "ctxT": cTt,
                "ones": ones_t.astype(BF16),
                "wq": wq8[:, s].astype(BF16),
                "wk": wk8[:, s].astype(BF16),
                "wv": wv8[:, s].astype(BF16),
                "wo": np.ascontiguousarray(wo8[s, :]).astype(BF16),
            }
        )
    return m_pad, in_maps


def kernel(x, context, mask, Wq, Wk, Wv, Wo, bo):
    from concourse.bass_utils import run_bass_kernel_spmd

    m_pad, in_maps = make_in_maps(x, context, mask, Wq, Wk, Wv, Wo)
    nc = _get_program(m_pad)
    res = run_bass_kernel_spmd(nc, in_maps, core_ids=list(range(8))).results
    out = np.empty((B, N, DIM), dtype=np.float32)
    bo32 = np.asarray(bo, dtype=np.float32)
    for b in range(B):
        out[b] = res[2 * b]["out"] + res[2 * b + 1]["out"] + bo32
    return out
